# revision 1
# baseline (speedup 1.0000x reference)
"""Trainium2 Bass kernel for nn_AttentionHiddenFusion_37048387895870.

Full-input contract: kernel(**inputs) takes the unsharded tensors from
setup_inputs() and returns the full [16, 4096, 1024] float32 output.

Strategy: pure data-parallel over the batch dim — each of the 8 NeuronCores
gets 2 batches (its CLS rows stay with the shard).  Per core, the layernorm
is folded into a 16-column projection matmul (W_cat = [gamma*Wr.T,
gamma*Wtg, ones, gamma*Wc.T]) over DMA-transposed bf16 activations, so the
normalized attention tensor is never materialized; sumsq comes from a
Square-activation with per-partition accumulation; rsqrt is a bitcast
Newton iteration on the vector engine; gelu/tanh run on the scalar engine
(all in the single `gelu_and_others` table set — sigmoid is expressed via
tanh); and the final update is two fused scalar_tensor_tensor ops:
out = hidden * (1 + gate*token_gate * layer_scale * tanh(gelu(TH) @ We.T)).

The update term is ~2.6e-6 of |hidden|, so the bf16 compute path keeps the
end-to-end relative error at ~5e-8; `hidden` itself flows through in fp32.
"""
import sys

sys.path.insert(0, '/opt/trn_rl_repo')

import numpy as np
import ml_dtypes
import concourse.bass as bass
import concourse.mybir as mybir
import concourse.tile as tile
from concourse import bacc
from concourse.bass_utils import run_bass_kernel_spmd

F32, BF16, I32 = mybir.dt.float32, mybir.dt.bfloat16, mybir.dt.int32
AF = mybir.ActivationFunctionType
ALU = mybir.AluOpType
AX = mybir.AxisListType
MAGIC = np.uint32(0x5F3759DF)

B, S, H, A, D = 16, 4096, 1024, 1024, 6
NCORES = 8
BLOC = B // NCORES


def build_nc(BLOC=2, S=4096, A=1024, H=1024, D=6, G=8, MB=4, MBH=None, reps=1, loop_reps=0, sq_pool_frac=0.0, stt2_pool_frac=0.0, abf_bufs=2, big_bufs=3, tr_pe=0.0, pmm_bufs=2, tcopy_act_frac=0.0, out_sp_frac=0.0, attn_hwdge=0, sm_bufs=None):
    MBH = MB if MBH is None else MBH
    T = BLOC * S
    NT = T // 128
    TPB = S // 128
    NCH = A // 128
    assert NT % G == 0 and G % MB == 0 and G % MBH == 0

    nc = bacc.Bacc("TRN2", target_bir_lowering=False, debug=False)
    attn = nc.dram_tensor("attn", [T, A], F32, kind="ExternalInput")
    hid = nc.dram_tensor("hid", [T, H], F32, kind="ExternalInput")
    wcat = nc.dram_tensor("wcat", [128, NCH * 16], BF16, kind="ExternalInput")
    wet = nc.dram_tensor("wet", [D, H], BF16, kind="ExternalInput")
    lsd = nc.dram_tensor("lsd", [1, H], BF16, kind="ExternalInput")
    ident = nc.dram_tensor("ident", [128, 128], BF16, kind="ExternalInput")
    rr = nc.dram_tensor("rr", [1, 16], F32, kind="ExternalInput")
    cconst = nc.dram_tensor("cconst", [1, 16], F32, kind="ExternalInput")
    wsg = nc.dram_tensor("wsg", [1, D], F32, kind="ExternalInput")
    out = nc.dram_tensor("out", [T, H], F32, kind="ExternalOutput")

    attn_b = attn.rearrange("(b s) a -> b s a", s=S)
    # [tile, 128, A] views for macro loads
    attn_t = attn.rearrange("(n p) a -> p n a", p=128)
    hid_t = hid.rearrange("(n p) a -> p n a", p=128)
    out_t = out.rearrange("(n p) a -> p n a", p=128)

    with tile.TileContext(nc) as tc, \
         tc.tile_pool(name="consts", bufs=1) as cpool, \
         tc.tile_pool(name="abf", bufs=abf_bufs) as abf_pool, \
         tc.tile_pool(name="aT", bufs=abf_bufs) as aT_pool, \
         tc.tile_pool(name="sq", bufs=2) as sq_pool, \
         tc.tile_pool(name="hidp", bufs=2) as hid_pool, \
         tc.tile_pool(name="outp", bufs=2) as out_pool, \
         tc.tile_pool(name="big2", bufs=big_bufs) as big2_pool, \
         tc.tile_pool(name="smalls", bufs=(sm_bufs or 2 * G + 4)) as sm_pool, \
         tc.tile_pool(name="ps_st", bufs=2, space="PSUM") as ps_st, \
         tc.tile_pool(name="ps_tt", bufs=2, space="PSUM") as ps_tt, \
         tc.tile_pool(name="ps_mm", bufs=pmm_bufs, space="PSUM") as ps_mm, \
         tc.tile_pool(name="ps_aT", bufs=2, space="PSUM") as ps_aT:

        # ---- constants ----
        wcat_sb = cpool.tile([128, NCH * 16], BF16)
        nc.gpsimd.dma_start(wcat_sb[:], wcat[:, :])
        wet_sb = cpool.tile([D, H], BF16)
        nc.gpsimd.dma_start(wet_sb[:], wet[:, :])
        ls_sb = cpool.tile([128, H], BF16)
        nc.gpsimd.dma_start(ls_sb[:], lsd[:, :].to_broadcast((128, H)))
        id_sb = cpool.tile([128, 128], BF16)
        nc.gpsimd.dma_start(id_sb[:], ident[:, :])
        rr_sb = cpool.tile([128, 16], F32)
        nc.gpsimd.dma_start(rr_sb[:], rr[:, :].to_broadcast((128, 16)))
        cc_sb = cpool.tile([128, 16], F32)
        nc.gpsimd.dma_start(cc_sb[:], cconst[:, :].to_broadcast((128, 16)))
        wsg_sb = cpool.tile([128, D], F32)
        nc.gpsimd.dma_start(wsg_sb[:], wsg[:, :].to_broadcast((128, D)))
        ccb_sb = [cpool.tile([128, 16], F32, tag=f"ccb{b}", name=f"ccb{b}")
                  for b in range(BLOC)]

        def stats_chain(pst_ap, ssq_g, g):
            """pst_ap: [128, g, 16] P-stats view (PSUM ok) + ssq_g [128, g]."""
            mu_g = sm_pool.tile([128, g], F32, tag="mu")
            nc.vector.tensor_scalar(mu_g[:], pst_ap[:, :, 7], 1.0 / A, None,
                                    ALU.mult)
            nmu_g = sm_pool.tile([128, g], F32, tag="nmu")
            nc.vector.tensor_scalar(nmu_g[:], mu_g[:], -1.0, None, ALU.mult)
            var_g = sm_pool.tile([128, g], F32, tag="var")
            nc.vector.tensor_tensor(var_g[:], nmu_g[:], mu_g[:], ALU.mult)
            nc.vector.tensor_tensor(var_g[:], var_g[:], ssq_g[:], ALU.add)
            y0 = sm_pool.tile([128, g], F32, tag="y0")
            nc.vector.tensor_scalar(
                y0[:].bitcast(I32), var_g[:].bitcast(I32), 1, None,
                ALU.logical_shift_right)
            nc.vector.tensor_tensor(
                y0[:].bitcast(I32),
                rr_sb[:, 15:16].bitcast(I32).to_broadcast((128, g)),
                y0[:].bitcast(I32), ALU.subtract)
            t1 = sm_pool.tile([128, g], F32, tag="t1")
            nc.vector.tensor_tensor(t1[:], y0[:], y0[:], ALU.mult)
            nc.vector.tensor_tensor(t1[:], t1[:], var_g[:], ALU.mult)
            nc.vector.tensor_scalar(t1[:], t1[:], -0.5, 1.5, ALU.mult, ALU.add)
            s_g = sm_pool.tile([128, g], F32, tag="sg")
            nc.vector.tensor_tensor(s_g[:], t1[:], y0[:], ALU.mult)
            nsmu_g = sm_pool.tile([128, g], F32, tag="nsmu")
            nc.vector.tensor_tensor(nsmu_g[:], s_g[:], nmu_g[:], ALU.mult)
            return s_g, nsmu_g

        # ================= CLS stage =================
        cls_bf = abf_pool.tile([128, A], BF16, tag="clsbf")
        nc.vector.memset(cls_bf[:], 0.0)
        nc.gpsimd.dma_start(cls_bf[0:BLOC, :], attn_b[:, 0, :])
        clsT = aT_pool.tile([128, NCH, 128], BF16, tag="clsT")
        nc.sync.dma_start_transpose(clsT[:], cls_bf[:])
        cls_sq = sq_pool.tile([128, A], BF16, tag="sq")
        cls_ssq = sm_pool.tile([128, 1], F32, tag="clsssq")
        nc.scalar.activation(cls_sq[:], cls_bf[:], AF.Square, scale=1.0 / 32.0,
                             accum_out=cls_ssq[:])
        pcls = ps_st.tile([128, G * 16], F32, tag="pst")
        for k in range(NCH):
            nc.tensor.matmul(pcls[:, 0:16], clsT[:, k, :],
                             wcat_sb[:, k * 16:k * 16 + 16],
                             start=(k == 0), stop=(k == NCH - 1))
        s_c, nsmu_c = stats_chain(
            pcls[:, 0:16].rearrange("p (g c) -> p g c", c=16), cls_ssq, 1)
        th2c = sm_pool.tile([128, 16], F32, tag="th2c")
        nc.vector.tensor_scalar(th2c[:, 0:15], pcls[:, 0:15], s_c[:], None,
                                ALU.mult)
        nc.vector.scalar_tensor_tensor(th2c[:, 0:15], rr_sb[:, 0:15], nsmu_c[:],
                                       th2c[:, 0:15], ALU.mult, ALU.add)
        bc = sm_pool.tile([128, 16], F32, tag="bc")
        nc.vector.tensor_copy(bc[0:BLOC, :], cc_sb[0:BLOC, :])
        nc.vector.tensor_tensor(bc[0:BLOC, 0:6], bc[0:BLOC, 0:6],
                                th2c[0:BLOC, 8:14], ALU.add)
        nc.vector.tensor_tensor(bc[0:BLOC, 0:6], bc[0:BLOC, 0:6],
                                cc_sb[0:BLOC, 8:14], ALU.add)
        thc = sm_pool.tile([128, 6], F32, tag="thc")
        nc.vector.tensor_tensor(thc[0:BLOC, :], th2c[0:BLOC, 0:6],
                                bc[0:BLOC, 0:6], ALU.add)
        nc.scalar.activation(thc[0:BLOC, :], thc[0:BLOC, :], AF.Gelu)
        zb = sm_pool.tile([128, 1], F32, tag="zb")
        nc.vector.tensor_tensor(thc[0:BLOC, :], thc[0:BLOC, :],
                                wsg_sb[0:BLOC, :], ALU.mult)
        nc.vector.reduce_sum(zb[0:BLOC, :], thc[0:BLOC, :], axis=AX.X)
        nc.scalar.activation(zb[0:BLOC, :], zb[0:BLOC, :], AF.Tanh, scale=0.5,
                             bias=cc_sb[0:BLOC, 7:8])
        nc.vector.tensor_scalar(bc[0:BLOC, 7:8], zb[0:BLOC, :], 0.25, 0.25,
                                ALU.mult, ALU.add)
        for b in range(BLOC):
            bc0 = sm_pool.tile([1, 16], F32, tag=f"bc0_{b}", name=f"bc0_{b}")
            nc.sync.dma_start(bc0[:], bc[b:b + 1, :])
            nc.gpsimd.partition_broadcast(ccb_sb[b][:], bc0[:])

        # ================= token tiles =================
        import contextlib
        loop_cm = (tc.For_i(0, loop_reps, 1,
                            hint_engines=tuple(nc.engines.keys()))
                   if loop_reps else contextlib.nullcontext())
        with loop_cm:
         for rep in range(reps):
          for grp in range(NT // G):
             pst = ps_st.tile([128, G * 16], F32, tag="pst")
             pst3 = pst[:].rearrange("p (g c) -> p g c", c=16)
             ssq_g = sm_pool.tile([128, G], F32, tag="ssqg")
             for m in range(G // MB):
                 t0 = grp * G + m * MB
                 abf = abf_pool.tile([128, MB * A], BF16, tag="abf")
                 nc.gpsimd.dma_start(
                     abf[:].rearrange("p (n a) -> p n a", n=MB),
                     attn_t[:, t0:t0 + MB, :])
                 aT = aT_pool.tile([128, MB * NCH, 128], BF16, tag="aT")
                 nc.sync.dma_start_transpose(aT[:], abf[:])
                 for i in range(MB):
                     gi = m * MB + i
                     sq = sq_pool.tile([128, A], BF16, tag="sq")
                     nc.scalar.activation(sq[:], abf[:, i * A:(i + 1) * A],
                                          AF.Square, scale=1.0 / 32.0,
                                          accum_out=ssq_g[:, gi:gi + 1])
                     for k in range(NCH):
                         nc.tensor.matmul(pst[:, gi * 16:gi * 16 + 16],
                                          aT[:, i * NCH + k, :],
                                          wcat_sb[:, k * 16:k * 16 + 16],
                                          start=(k == 0), stop=(k == NCH - 1))
             s_g, nsmu_g = stats_chain(pst3, ssq_g, G)
             hd = [None] * (G // MBH)
             for m in range(G // MBH):
                 t0 = grp * G + m * MBH
                 hd[m] = hid_pool.tile([128, MBH * H], F32, tag="hd",
                                       name=f"hd{rep}_{grp}_{m}")
                 nc.sync.dma_start(
                     hd[m][:].rearrange("p (n a) -> p n a", n=MBH),
                     hid_t[:, t0:t0 + MBH, :])
             of = [None] * (G // MBH)
             for m in range(G // MBH):
                 of[m] = out_pool.tile([128, MBH * H], F32, tag="of",
                                       name=f"of{rep}_{grp}_{m}")
             for i in range(G):
                 ti = grp * G + i
                 b = ti // TPB
                 m, iv = i // MBH, i % MBH
                 s_i = s_g[:, i:i + 1]
                 nsmu_i = nsmu_g[:, i:i + 1]
                 th = sm_pool.tile([128, 7], F32, tag="th")
                 nc.vector.tensor_scalar(th[:], pst[:, i * 16:i * 16 + 7],
                                         s_i, None, ALU.mult)
                 nc.vector.scalar_tensor_tensor(th[:], rr_sb[:, 0:7], nsmu_i,
                                                th[:], ALU.mult, ALU.add)
                 thg = sm_pool.tile([128, 8], BF16, tag="thg")
                 nc.vector.tensor_tensor(th[:, 0:6], th[:, 0:6],
                                         ccb_sb[b][:, 0:6], ALU.add)
                 nc.scalar.activation(thg[:, 0:6], th[:, 0:6], AF.Gelu)
                 nc.vector.memset(thg[:, 6:8], 0.0)
                 gcol = sm_pool.tile([128, 1], F32, tag="gcol")
                 nc.scalar.activation(gcol[:], th[:, 6:7], AF.Tanh, scale=0.5,
                                      bias=ccb_sb[b][:, 6:7])
                 nc.vector.tensor_scalar(gcol[:], gcol[:], ccb_sb[b][:, 7:8],
                                         ccb_sb[b][:, 7:8], ALU.mult, ALU.add)
                 ptt = ps_tt.tile([8, 128], BF16, tag="ptt")
                 nc.tensor.transpose(ptt[:], thg[:], id_sb[:])
                 thT = sm_pool.tile([8, 128], BF16, tag="thT")
                 nc.vector.tensor_copy(thT[:], ptt[:])
                 pmm = ps_mm.tile([128, H], F32, tag="pmm")
                 for j in range(H // 512):
                     nc.tensor.matmul(pmm[:, j * 512:(j + 1) * 512], thT[0:D, :],
                                      wet_sb[:, j * 512:(j + 1) * 512],
                                      start=True, stop=True)
                 tnh = big2_pool.tile([128, H], BF16, tag="tnh")
                 nc.scalar.activation(tnh[:], pmm[:], AF.Tanh)
                 u = big2_pool.tile([128, H], BF16, tag="u")
                 nc.vector.scalar_tensor_tensor(u[:], tnh[:], gcol[:], ls_sb[:],
                                                ALU.mult, ALU.mult)
                 nc.vector.scalar_tensor_tensor(
                     of[m][:, iv * H:(iv + 1) * H], u[:], 1.0,
                     hd[m][:, iv * H:(iv + 1) * H], ALU.add, ALU.mult)
             for m in range(G // MBH):
                 t0 = grp * G + m * MBH
                 nc.scalar.dma_start(
                     out_t[:, t0:t0 + MBH, :],
                     of[m][:].rearrange("p (n a) -> p n a", n=MBH))

    nc.compile()
    return nc


def host_params(p):
    """Precompute the folded parameter images (identical on every core)."""
    gamma = p["ln_gamma"].astype(np.float64)
    beta = p["ln_beta"].astype(np.float64)
    Wr = p["Wr"].astype(np.float64)
    Wtg = p["Wtg"].astype(np.float64)
    Wc = p["Wc"].astype(np.float64)
    We = p["We"].astype(np.float64)
    Wsg = p["Wsg"].astype(np.float64)
    btg = float(np.asarray(p["btg"]).reshape(-1)[0])
    bsg = float(np.asarray(p["bsg"]).reshape(-1)[0])
    ls = p["layer_scale"].astype(np.float64).reshape(H)

    wcat_full = np.zeros((A, 16), np.float64)
    wcat_full[:, 0:6] = (gamma[None, :] * Wr).T
    wcat_full[:, 6] = gamma * Wtg[0]
    wcat_full[:, 7] = 1.0
    wcat_full[:, 8:14] = (gamma[None, :] * Wc).T
    wcat_img = wcat_full.reshape(8, 128, 16).transpose(1, 0, 2).reshape(128, 128)

    rr = np.zeros((1, 16), np.float32)
    rr[0, 0:6] = (gamma[None, :] * Wr).sum(axis=1)
    rr[0, 6] = (gamma * Wtg[0]).sum()
    rr[0, 8:14] = (gamma[None, :] * Wc).sum(axis=1)
    rr[0, 15] = MAGIC.view(np.float32)

    cconst = np.zeros((1, 16), np.float32)
    cconst[0, 0:6] = (beta[None, :] * Wr).sum(axis=1)
    cconst[0, 6] = 0.5 * ((beta * Wtg[0]).sum() + btg)
    cconst[0, 7] = 0.5 * bsg
    cconst[0, 8:14] = (beta[None, :] * Wc).sum(axis=1)

    return {
        "wcat": wcat_img.astype(ml_dtypes.bfloat16),
        "wet": We.T.astype(ml_dtypes.bfloat16),
        "lsd": ls.reshape(1, H).astype(ml_dtypes.bfloat16),
        "ident": np.eye(128, dtype=ml_dtypes.bfloat16),
        "rr": rr,
        "cconst": cconst,
        "wsg": Wsg.reshape(1, D).astype(np.float32),
    }


BEST_CFG = dict(G=16, MB=8, MBH=4, sq_pool_frac=0.5, stt2_pool_frac=0.25)

_CACHE = {}


def _get_nc():
    if "nc" not in _CACHE:
        _CACHE["nc"] = build_nc(BLOC=BLOC, S=S, A=A, H=H, D=D, **BEST_CFG)
    return _CACHE["nc"]


def kernel(hidden, attn_out, ln_gamma, ln_beta, Wr, Wc, We, Wtg, btg, Wsg, bsg,
           layer_scale, _trace=False):
    nc = _get_nc()
    consts = host_params({
        "ln_gamma": np.asarray(ln_gamma), "ln_beta": np.asarray(ln_beta),
        "Wr": np.asarray(Wr), "Wc": np.asarray(Wc), "We": np.asarray(We),
        "Wtg": np.asarray(Wtg), "btg": np.asarray(btg),
        "Wsg": np.asarray(Wsg), "bsg": np.asarray(bsg),
        "layer_scale": np.asarray(layer_scale),
    })
    hidden = np.ascontiguousarray(np.asarray(hidden, dtype=np.float32))
    attn_out = np.ascontiguousarray(np.asarray(attn_out, dtype=np.float32))
    in_maps = []
    for c in range(NCORES):
        in_maps.append({
            "attn": attn_out[c * BLOC:(c + 1) * BLOC].reshape(BLOC * S, A),
            "hid": hidden[c * BLOC:(c + 1) * BLOC].reshape(BLOC * S, H),
            **consts,
        })
    res = run_bass_kernel_spmd(nc, in_maps, core_ids=list(range(NCORES)),
                               trace=_trace)
    out = np.empty((B, S, H), np.float32)
    for c in range(NCORES):
        out[c * BLOC:(c + 1) * BLOC] = res.results[c]["out"].reshape(BLOC, S, H)
    if _trace:
        return out, res
    return out



# revision 2
# speedup vs baseline: 1.6023x; 1.6023x over previous
"""Trainium2 Bass kernel for nn_AttentionHiddenFusion_37048387895870.

Full-input contract: kernel(**inputs) takes the unsharded tensors from
setup_inputs() and returns the full [16, 4096, 1024] float32 output.

Strategy: pure data-parallel over the batch dim — each of the 8 NeuronCores
gets 2 batches (its CLS rows stay with the shard).  Per core, the layernorm
is folded into a 16-column projection matmul (W_cat = [gamma*Wr.T,
gamma*Wtg, ones, gamma*Wc.T]); activations ship as bf16 and `hidden`/`out`
as fp16, so per-core HBM traffic is 16.8+16.8+16.8 MB (the update term is
~2.6e-6 of |hidden|, so 16-bit transport keeps rel err ~2e-4, far inside
the 2e-2 gate).  attn tiles are transposed on the PE (identity matmuls into
PSUM + DVE copy-back) instead of the DMA xbar, and loads go through HWDGE
(sync engine) — SWDGE descriptor generation was ~100us/core.  tanh(z)~=z
(|z|~3e-3, rel err 3e-6 on the update), so layer_scale folds into We and
the per-token gate folds into the tiny [128,6] gelu output before the
H-matmul; the final update is a single fused (pmm+1)*hid DVE op reading
PSUM.  Per-group [128,G,*] broadcast-AP ops replace ~128 small per-tile
vector instructions.  sumsq comes from an ACT Square with per-partition
accumulation; rsqrt is a bitcast Newton iteration on the vector engine;
gelu/tanh/copy all live in the single `gelu_and_others` table (sigmoid is
expressed via tanh).
"""
import sys

sys.path.insert(0, '/opt/trn_rl_repo')

import numpy as np
import ml_dtypes
import concourse.bass as bass
import concourse.mybir as mybir
import concourse.tile as tile
from concourse import bacc
from concourse.bass_utils import run_bass_kernel_spmd

F32, BF16, I32 = mybir.dt.float32, mybir.dt.bfloat16, mybir.dt.int32
AF = mybir.ActivationFunctionType
ALU = mybir.AluOpType
AX = mybir.AxisListType
MAGIC = np.uint32(0x5F3759DF)

B, S, H, A, D = 16, 4096, 1024, 1024, 6
NCORES = 8
BLOC = B // NCORES


def build_nc(BLOC=2, S=4096, A=1024, H=1024, D=6, G=8, MB=4, MBH=None, reps=1, loop_reps=0, sq_pool_frac=0.0, stt2_pool_frac=0.0, abf_bufs=2, big_bufs=3, tr_pe=0.0, pmm_bufs=2, tcopy_act_frac=0.0, out_sp_frac=0.0, attn_hwdge=0, sm_bufs=None, tr='dma', ce='alt', out16=0, a16=0, paT_bufs=2, fuse=0, sqe='dve', ofe='dve', h16=0, pmm16=0, pstt_bufs=2, psst_bufs=2, gw=0):
    MBH = MB if MBH is None else MBH
    T = BLOC * S
    NT = T // 128
    TPB = S // 128
    NCH = A // 128
    assert NT % G == 0 and G % MB == 0 and G % MBH == 0

    nc = bacc.Bacc("TRN2", target_bir_lowering=False, debug=False)
    F16 = mybir.dt.float16
    ADT = BF16 if a16 else F32
    ODT = F16 if out16 else F32
    attn = nc.dram_tensor("attn", [T, A], ADT, kind="ExternalInput")
    HDT = mybir.dt.float16 if h16 else F32
    hid = nc.dram_tensor("hid", [T, H], HDT, kind="ExternalInput")
    wcat = nc.dram_tensor("wcat", [128, NCH * 16], BF16, kind="ExternalInput")
    wet = nc.dram_tensor("wet", [D, H], BF16, kind="ExternalInput")
    lsd = nc.dram_tensor("lsd", [1, H], BF16, kind="ExternalInput")
    ident = nc.dram_tensor("ident", [128, 128], BF16, kind="ExternalInput")
    rr = nc.dram_tensor("rr", [1, 16], F32, kind="ExternalInput")
    cconst = nc.dram_tensor("cconst", [1, 16], F32, kind="ExternalInput")
    wsg = nc.dram_tensor("wsg", [1, D], F32, kind="ExternalInput")
    out = nc.dram_tensor("out", [T, H], ODT, kind="ExternalOutput")

    attn_b = attn.rearrange("(b s) a -> b s a", s=S)
    # [tile, 128, A] views for macro loads
    attn_t = attn.rearrange("(n p) a -> p n a", p=128)
    hid_t = hid.rearrange("(n p) a -> p n a", p=128)
    out_t = out.rearrange("(n p) a -> p n a", p=128)

    with tile.TileContext(nc) as tc, \
         tc.tile_pool(name="consts", bufs=1) as cpool, \
         tc.tile_pool(name="abf", bufs=abf_bufs) as abf_pool, \
         tc.tile_pool(name="aT", bufs=abf_bufs) as aT_pool, \
         tc.tile_pool(name="sq", bufs=2) as sq_pool, \
         tc.tile_pool(name="hidp", bufs=2) as hid_pool, \
         tc.tile_pool(name="outp", bufs=2) as out_pool, \
         tc.tile_pool(name="big2", bufs=big_bufs) as big2_pool, \
         tc.tile_pool(name="smalls", bufs=(sm_bufs or 2 * G + 4)) as sm_pool, \
         tc.tile_pool(name="gwp", bufs=2) as gw_pool, \
         tc.tile_pool(name="ps_st", bufs=psst_bufs, space="PSUM") as ps_st, \
         tc.tile_pool(name="ps_tt", bufs=pstt_bufs, space="PSUM") as ps_tt, \
         tc.tile_pool(name="ps_mm", bufs=pmm_bufs, space="PSUM") as ps_mm, \
         tc.tile_pool(name="ps_aT", bufs=paT_bufs, space="PSUM") as ps_aT:

        # ---- constants ----
        wcat_sb = cpool.tile([128, NCH * 16], BF16)
        nc.gpsimd.dma_start(wcat_sb[:], wcat[:, :])
        wet_sb = cpool.tile([D, H], BF16)
        nc.gpsimd.dma_start(wet_sb[:], wet[:, :])
        ls_sb = None
        if not fuse:
            ls_sb = cpool.tile([128, H], BF16)
            nc.gpsimd.dma_start(ls_sb[:], lsd[:, :].to_broadcast((128, H)))
        id_sb = cpool.tile([128, 128], BF16)
        nc.gpsimd.dma_start(id_sb[:], ident[:, :])
        rr_sb = cpool.tile([128, 16], F32)
        nc.gpsimd.dma_start(rr_sb[:], rr[:, :].to_broadcast((128, 16)))
        cc_sb = cpool.tile([128, 16], F32)
        nc.gpsimd.dma_start(cc_sb[:], cconst[:, :].to_broadcast((128, 16)))
        wsg_sb = cpool.tile([128, D], F32)
        nc.gpsimd.dma_start(wsg_sb[:], wsg[:, :].to_broadcast((128, D)))
        ccb_sb = [cpool.tile([128, 16], F32, tag=f"ccb{b}", name=f"ccb{b}")
                  for b in range(BLOC)]

        def stats_chain(pst_ap, ssq_g, g):
            """pst_ap: [128, g, 16] P-stats view (PSUM ok) + ssq_g [128, g]."""
            mu_g = sm_pool.tile([128, g], F32, tag="mu")
            nc.vector.tensor_scalar(mu_g[:], pst_ap[:, :, 7], 1.0 / A, None,
                                    ALU.mult)
            nmu_g = sm_pool.tile([128, g], F32, tag="nmu")
            nc.vector.tensor_scalar(nmu_g[:], mu_g[:], -1.0, None, ALU.mult)
            var_g = sm_pool.tile([128, g], F32, tag="var")
            nc.vector.tensor_tensor(var_g[:], nmu_g[:], mu_g[:], ALU.mult)
            nc.vector.tensor_tensor(var_g[:], var_g[:], ssq_g[:], ALU.add)
            y0 = sm_pool.tile([128, g], F32, tag="y0")
            nc.vector.tensor_scalar(
                y0[:].bitcast(I32), var_g[:].bitcast(I32), 1, None,
                ALU.logical_shift_right)
            nc.vector.tensor_tensor(
                y0[:].bitcast(I32),
                rr_sb[:, 15:16].bitcast(I32).to_broadcast((128, g)),
                y0[:].bitcast(I32), ALU.subtract)
            t1 = sm_pool.tile([128, g], F32, tag="t1")
            nc.vector.tensor_tensor(t1[:], y0[:], y0[:], ALU.mult)
            nc.vector.tensor_tensor(t1[:], t1[:], var_g[:], ALU.mult)
            nc.vector.tensor_scalar(t1[:], t1[:], -0.5, 1.5, ALU.mult, ALU.add)
            s_g = sm_pool.tile([128, g], F32, tag="sg")
            nc.vector.tensor_tensor(s_g[:], t1[:], y0[:], ALU.mult)
            nsmu_g = sm_pool.tile([128, g], F32, tag="nsmu")
            nc.vector.tensor_tensor(nsmu_g[:], s_g[:], nmu_g[:], ALU.mult)
            return s_g, nsmu_g

        # ================= CLS stage =================
        cls_bf = abf_pool.tile([128, A], BF16, tag="clsbf")
        nc.vector.memset(cls_bf[:], 0.0)
        nc.gpsimd.dma_start(cls_bf[0:BLOC, :], attn_b[:, 0, :])
        clsT = aT_pool.tile([128, NCH, 128], BF16, tag="clsT")
        nc.sync.dma_start_transpose(clsT[:], cls_bf[:])
        cls_sq = sq_pool.tile([128, A], BF16, tag="sq")
        cls_ssq = sm_pool.tile([128, 1], F32, tag="clsssq")
        nc.scalar.activation(cls_sq[:], cls_bf[:], AF.Square, scale=1.0 / 32.0,
                             accum_out=cls_ssq[:])
        pcls = ps_st.tile([128, G * 16], F32, tag="pst")
        for k in range(NCH):
            nc.tensor.matmul(pcls[:, 0:16], clsT[:, k, :],
                             wcat_sb[:, k * 16:k * 16 + 16],
                             start=(k == 0), stop=(k == NCH - 1))
        s_c, nsmu_c = stats_chain(
            pcls[:, 0:16].rearrange("p (g c) -> p g c", c=16), cls_ssq, 1)
        th2c = sm_pool.tile([128, 16], F32, tag="th2c")
        nc.vector.tensor_scalar(th2c[:, 0:15], pcls[:, 0:15], s_c[:], None,
                                ALU.mult)
        nc.vector.scalar_tensor_tensor(th2c[:, 0:15], rr_sb[:, 0:15], nsmu_c[:],
                                       th2c[:, 0:15], ALU.mult, ALU.add)
        bc = sm_pool.tile([128, 16], F32, tag="bc")
        nc.vector.tensor_copy(bc[0:BLOC, :], cc_sb[0:BLOC, :])
        nc.vector.tensor_tensor(bc[0:BLOC, 0:6], bc[0:BLOC, 0:6],
                                th2c[0:BLOC, 8:14], ALU.add)
        nc.vector.tensor_tensor(bc[0:BLOC, 0:6], bc[0:BLOC, 0:6],
                                cc_sb[0:BLOC, 8:14], ALU.add)
        thc = sm_pool.tile([128, 6], F32, tag="thc")
        nc.vector.tensor_tensor(thc[0:BLOC, :], th2c[0:BLOC, 0:6],
                                bc[0:BLOC, 0:6], ALU.add)
        nc.scalar.activation(thc[0:BLOC, :], thc[0:BLOC, :], AF.Gelu)
        zb = sm_pool.tile([128, 1], F32, tag="zb")
        nc.vector.tensor_tensor(thc[0:BLOC, :], thc[0:BLOC, :],
                                wsg_sb[0:BLOC, :], ALU.mult)
        nc.vector.reduce_sum(zb[0:BLOC, :], thc[0:BLOC, :], axis=AX.X)
        nc.scalar.activation(zb[0:BLOC, :], zb[0:BLOC, :], AF.Tanh, scale=0.5,
                             bias=cc_sb[0:BLOC, 7:8])
        nc.vector.tensor_scalar(bc[0:BLOC, 7:8], zb[0:BLOC, :], 0.25, 0.25,
                                ALU.mult, ALU.add)
        for b in range(BLOC):
            bc0 = sm_pool.tile([1, 16], F32, tag=f"bc0_{b}", name=f"bc0_{b}")
            nc.sync.dma_start(bc0[:], bc[b:b + 1, :])
            nc.gpsimd.partition_broadcast(ccb_sb[b][:], bc0[:])

        # ================= token tiles =================
        import contextlib
        loop_cm = (tc.For_i(0, loop_reps, 1,
                            hint_engines=tuple(nc.engines.keys()))
                   if loop_reps else contextlib.nullcontext())
        with loop_cm:
         for rep in range(reps):
          for grp in range(NT // G):
             pst = ps_st.tile([128, G * 16], F32, tag="pst")
             pst3 = pst[:].rearrange("p (g c) -> p g c", c=16)
             ssq_g = sm_pool.tile([128, G], F32, tag="ssqg")
             for m in range(G // MB):
                 t0 = grp * G + m * MB
                 abf = abf_pool.tile([128, MB * A], BF16, tag="abf")
                 a_eng = (nc.sync if attn_hwdge == 1 else
                          nc.scalar if attn_hwdge == 2 else nc.gpsimd)
                 a_eng.dma_start(
                     abf[:].rearrange("p (n a) -> p n a", n=MB),
                     attn_t[:, t0:t0 + MB, :])
                 aT = aT_pool.tile([128, MB * NCH, 128], BF16, tag="aT")
                 if tr == 'dma':
                     nc.sync.dma_start_transpose(aT[:], abf[:])
                 else:
                     for i in range(MB):
                         pat = ps_aT.tile([128, NCH * 128], BF16, tag="pat")
                         for k in range(NCH):
                             nc.tensor.transpose(
                                 pat[:, k * 128:(k + 1) * 128],
                                 abf[:, i * A + k * 128:i * A + (k + 1) * 128],
                                 id_sb[:])
                         dst = aT[:, i * NCH:(i + 1) * NCH, :].rearrange(
                             "p n a -> p (n a)")
                         use_act = (ce == 'act' or (ce == 'alt' and i % 2 == 0) or (ce == 'q4' and i % 4 == 0))
                         if ce == 'pool':
                             nc.gpsimd.tensor_copy(dst, pat[:])
                         elif use_act:
                             nc.scalar.activation(dst, pat[:], AF.Copy)
                         else:
                             nc.vector.tensor_copy(dst, pat[:])
                 for i in range(MB):
                     gi = m * MB + i
                     sq = sq_pool.tile([128, A], BF16, tag="sq")
                     if fuse and sqe in ('dve', 'pool'):
                         sq_eng = nc.vector if sqe == 'dve' else nc.gpsimd
                         sq_eng.tensor_tensor_reduce(
                             sq[:], abf[:, i * A:(i + 1) * A],
                             abf[:, i * A:(i + 1) * A], 1.0 / A, 0.0,
                             ALU.mult, ALU.add,
                             accum_out=ssq_g[:, gi:gi + 1])
                     else:
                         nc.scalar.activation(sq[:], abf[:, i * A:(i + 1) * A],
                                              AF.Square, scale=1.0 / 32.0,
                                              accum_out=ssq_g[:, gi:gi + 1])
                     for k in range(NCH):
                         nc.tensor.matmul(pst[:, gi * 16:gi * 16 + 16],
                                          aT[:, i * NCH + k, :],
                                          wcat_sb[:, k * 16:k * 16 + 16],
                                          start=(k == 0), stop=(k == NCH - 1))
             s_g, nsmu_g = stats_chain(pst3, ssq_g, G)
             hd = [None] * (G // MBH)
             for m in range(G // MBH):
                 t0 = grp * G + m * MBH
                 hd[m] = hid_pool.tile([128, MBH * H], HDT, tag="hd",
                                       name=f"hd{rep}_{grp}_{m}")
                 nc.sync.dma_start(
                     hd[m][:].rearrange("p (n a) -> p n a", n=MBH),
                     hid_t[:, t0:t0 + MBH, :])
             of = [None] * (G // MBH)
             for m in range(G // MBH):
                 of[m] = out_pool.tile([128, MBH * H], ODT, tag="of",
                                       name=f"of{rep}_{grp}_{m}")
             if fuse and gw:
                 b0 = (grp * G) // TPB
                 tha = gw_pool.tile([128, G, 8], F32, tag="tha")
                 nc.vector.tensor_tensor(
                     tha[:, :, 0:7], pst3[:, :, 0:7],
                     s_g[:].to_broadcast((128, G, 7)), ALU.mult)
                 tmp7 = gw_pool.tile([128, G, 7], F32, tag="tmp7")
                 nc.vector.tensor_tensor(
                     tmp7[:],
                     rr_sb[:, 0:7].rearrange("p (g c) -> p g c", g=1)
                     .to_broadcast((128, G, 7)),
                     nsmu_g[:].to_broadcast((128, G, 7)), ALU.mult)
                 nc.vector.tensor_tensor(tha[:, :, 0:7], tha[:, :, 0:7],
                                         tmp7[:], ALU.add)
                 nc.vector.tensor_tensor(
                     tha[:, :, 0:6], tha[:, :, 0:6],
                     ccb_sb[b0][:, 0:6].rearrange("p (g c) -> p g c", g=1)
                     .to_broadcast((128, G, 6)), ALU.add)
                 thga = gw_pool.tile([128, G, 6], BF16, tag="thga")
                 nc.scalar.activation(thga[:], tha[:, :, 0:6], AF.Gelu)
                 gca = gw_pool.tile([128, G], F32, tag="gca")
                 nc.scalar.activation(gca[:], tha[:, :, 6], AF.Tanh, scale=0.5,
                                      bias=ccb_sb[b0][:, 6:7])
                 nc.vector.tensor_scalar(gca[:], gca[:], ccb_sb[b0][:, 7:8],
                                         ccb_sb[b0][:, 7:8], ALU.mult, ALU.add)
                 nc.vector.tensor_tensor(
                     thga[:], thga[:],
                     gca[:].to_broadcast((128, G, 6)), ALU.mult)
                 for i in range(G):
                     m, iv = i // MBH, i % MBH
                     ptt = ps_tt.tile([8, 128], BF16, tag="ptt")
                     nc.tensor.transpose(ptt[0:6, :], thga[:, i, :], id_sb[:])
                     thT = sm_pool.tile([8, 128], BF16, tag="thT")
                     nc.vector.tensor_copy(thT[0:6, :], ptt[0:6, :])
                     for j in range(H // 512):
                         pmm = ps_mm.tile([128, 512], F32, tag="pmm")
                         nc.tensor.matmul(pmm[:], thT[0:D, :],
                                          wet_sb[:, j * 512:(j + 1) * 512],
                                          start=True, stop=True)
                         oslc = slice(iv * H + j * 512, iv * H + (j + 1) * 512)
                         if pmm16 == 2:
                             v = sq_pool.tile([128, 512], BF16, tag="vps")
                             nc.scalar.activation(v[:], pmm[:], AF.Copy)
                             nc.vector.scalar_tensor_tensor(
                                 of[m][:, oslc], v[:], 1.0, hd[m][:, oslc],
                                 ALU.add, ALU.mult)
                         else:
                             nc.vector.scalar_tensor_tensor(
                                 of[m][:, oslc], pmm[:], 1.0, hd[m][:, oslc],
                                 ALU.add, ALU.mult)
             else:
              for i in range(G):
                 ti = grp * G + i
                 b = ti // TPB
                 m, iv = i // MBH, i % MBH
                 s_i = s_g[:, i:i + 1]
                 nsmu_i = nsmu_g[:, i:i + 1]
                 th = sm_pool.tile([128, 7], F32, tag="th")
                 nc.vector.tensor_scalar(th[:], pst[:, i * 16:i * 16 + 7],
                                         s_i, None, ALU.mult)
                 nc.vector.scalar_tensor_tensor(th[:], rr_sb[:, 0:7], nsmu_i,
                                                th[:], ALU.mult, ALU.add)
                 thg = sm_pool.tile([128, 8], BF16, tag="thg")
                 nc.vector.tensor_tensor(th[:, 0:6], th[:, 0:6],
                                         ccb_sb[b][:, 0:6], ALU.add)
                 nc.scalar.activation(thg[:, 0:6], th[:, 0:6], AF.Gelu)
                 nc.vector.memset(thg[:, 6:8], 0.0)
                 gcol = sm_pool.tile([128, 1], F32, tag="gcol")
                 nc.scalar.activation(gcol[:], th[:, 6:7], AF.Tanh, scale=0.5,
                                      bias=ccb_sb[b][:, 6:7])
                 nc.vector.tensor_scalar(gcol[:], gcol[:], ccb_sb[b][:, 7:8],
                                         ccb_sb[b][:, 7:8], ALU.mult, ALU.add)
                 if fuse:
                     nc.vector.tensor_scalar(thg[:, 0:6], thg[:, 0:6],
                                             gcol[:], None, ALU.mult)
                 ptt = ps_tt.tile([8, 128], BF16, tag="ptt")
                 nc.tensor.transpose(ptt[:], thg[:], id_sb[:])
                 thT = sm_pool.tile([8, 128], BF16, tag="thT")
                 nc.vector.tensor_copy(thT[:], ptt[:])
                 if fuse:
                     for j in range(H // 512):
                         pmm = ps_mm.tile([128, 512], F32, tag="pmm")
                         nc.tensor.matmul(pmm[:], thT[0:D, :],
                                          wet_sb[:, j * 512:(j + 1) * 512],
                                          start=True, stop=True)
                         oslc = slice(iv * H + j * 512, iv * H + (j + 1) * 512)
                         if pmm16 == 2:
                             v = sq_pool.tile([128, 512], BF16, tag="vps")
                             nc.scalar.activation(v[:], pmm[:], AF.Copy)
                             nc.vector.scalar_tensor_tensor(
                                 of[m][:, oslc], v[:], 1.0, hd[m][:, oslc],
                                 ALU.add, ALU.mult)
                         else:
                             nc.vector.scalar_tensor_tensor(
                                 of[m][:, oslc], pmm[:], 1.0, hd[m][:, oslc],
                                 ALU.add, ALU.mult)
                 else:
                     tnh = big2_pool.tile([128, H], BF16, tag="tnh")
                     if tr == 'pe':
                         for j in range(H // 512):
                             pmm = ps_mm.tile([128, 512], F32, tag="pmm")
                             nc.tensor.matmul(pmm[:], thT[0:D, :],
                                              wet_sb[:, j * 512:(j + 1) * 512],
                                              start=True, stop=True)
                             nc.scalar.activation(tnh[:, j * 512:(j + 1) * 512],
                                                  pmm[:], AF.Tanh)
                     else:
                         pmm = ps_mm.tile([128, H], F32, tag="pmm")
                         for j in range(H // 512):
                             nc.tensor.matmul(pmm[:, j * 512:(j + 1) * 512],
                                              thT[0:D, :],
                                              wet_sb[:, j * 512:(j + 1) * 512],
                                              start=True, stop=True)
                         nc.scalar.activation(tnh[:], pmm[:], AF.Tanh)
                     u = big2_pool.tile([128, H], BF16, tag="u")
                     nc.vector.scalar_tensor_tensor(u[:], tnh[:], gcol[:],
                                                    ls_sb[:], ALU.mult, ALU.mult)
                     nc.vector.scalar_tensor_tensor(
                         of[m][:, iv * H:(iv + 1) * H], u[:], 1.0,
                         hd[m][:, iv * H:(iv + 1) * H], ALU.add, ALU.mult)
             for m in range(G // MBH):
                 t0 = grp * G + m * MBH
                 nc.scalar.dma_start(
                     out_t[:, t0:t0 + MBH, :],
                     of[m][:].rearrange("p (n a) -> p n a", n=MBH))

    nc.compile()
    return nc


def host_params(p, fuse=0):
    """Precompute the folded parameter images (identical on every core)."""
    gamma = p["ln_gamma"].astype(np.float64)
    beta = p["ln_beta"].astype(np.float64)
    Wr = p["Wr"].astype(np.float64)
    Wtg = p["Wtg"].astype(np.float64)
    Wc = p["Wc"].astype(np.float64)
    We = p["We"].astype(np.float64)
    Wsg = p["Wsg"].astype(np.float64)
    btg = float(np.asarray(p["btg"]).reshape(-1)[0])
    bsg = float(np.asarray(p["bsg"]).reshape(-1)[0])
    ls = p["layer_scale"].astype(np.float64).reshape(H)

    wcat_full = np.zeros((A, 16), np.float64)
    wcat_full[:, 0:6] = (gamma[None, :] * Wr).T
    wcat_full[:, 6] = gamma * Wtg[0]
    wcat_full[:, 7] = 1.0
    wcat_full[:, 8:14] = (gamma[None, :] * Wc).T
    wcat_img = wcat_full.reshape(8, 128, 16).transpose(1, 0, 2).reshape(128, 128)

    rr = np.zeros((1, 16), np.float32)
    rr[0, 0:6] = (gamma[None, :] * Wr).sum(axis=1)
    rr[0, 6] = (gamma * Wtg[0]).sum()
    rr[0, 8:14] = (gamma[None, :] * Wc).sum(axis=1)
    rr[0, 15] = MAGIC.view(np.float32)

    cconst = np.zeros((1, 16), np.float32)
    cconst[0, 0:6] = (beta[None, :] * Wr).sum(axis=1)
    cconst[0, 6] = 0.5 * ((beta * Wtg[0]).sum() + btg)
    cconst[0, 7] = 0.5 * bsg
    cconst[0, 8:14] = (beta[None, :] * Wc).sum(axis=1)

    return {
        "wcat": wcat_img.astype(ml_dtypes.bfloat16),
        "wet": ((We.T * ls[None, :]) if fuse else We.T).astype(
            ml_dtypes.bfloat16),
        "lsd": ls.reshape(1, H).astype(ml_dtypes.bfloat16),
        "ident": np.eye(128, dtype=ml_dtypes.bfloat16),
        "rr": rr,
        "cconst": cconst,
        "wsg": Wsg.reshape(1, D).astype(np.float32),
    }


BEST_CFG = dict(G=16, MB=8, MBH=4, sq_pool_frac=0.5, stt2_pool_frac=0.25,
                tr='pe', ce='dve', fuse=1, sqe='act', out16=1, a16=1,
                h16=1, attn_hwdge=1, gw=1)

_CACHE = {}


def _get_nc():
    if "nc" not in _CACHE:
        _CACHE["nc"] = build_nc(BLOC=BLOC, S=S, A=A, H=H, D=D, **BEST_CFG)
    return _CACHE["nc"]


def kernel(hidden, attn_out, ln_gamma, ln_beta, Wr, Wc, We, Wtg, btg, Wsg, bsg,
           layer_scale, _trace=False):
    nc = _get_nc()
    consts = host_params({
        "ln_gamma": np.asarray(ln_gamma), "ln_beta": np.asarray(ln_beta),
        "Wr": np.asarray(Wr), "Wc": np.asarray(Wc), "We": np.asarray(We),
        "Wtg": np.asarray(Wtg), "btg": np.asarray(btg),
        "Wsg": np.asarray(Wsg), "bsg": np.asarray(bsg),
        "layer_scale": np.asarray(layer_scale),
    }, fuse=BEST_CFG.get('fuse', 0))
    if BEST_CFG.get('h16', 0):
        hidden = np.asarray(hidden, dtype=np.float32).astype(np.float16)
    else:
        hidden = np.ascontiguousarray(np.asarray(hidden, dtype=np.float32))
    if BEST_CFG.get('a16', 0):
        attn_out = np.asarray(attn_out, dtype=np.float32).astype(
            ml_dtypes.bfloat16)
    else:
        attn_out = np.ascontiguousarray(np.asarray(attn_out, dtype=np.float32))
    in_maps = []
    for c in range(NCORES):
        in_maps.append({
            "attn": attn_out[c * BLOC:(c + 1) * BLOC].reshape(BLOC * S, A),
            "hid": hidden[c * BLOC:(c + 1) * BLOC].reshape(BLOC * S, H),
            **consts,
        })
    res = run_bass_kernel_spmd(nc, in_maps, core_ids=list(range(NCORES)),
                               trace=_trace)
    out = np.empty((B, S, H), np.float32)
    for c in range(NCORES):
        out[c * BLOC:(c + 1) * BLOC] = (
            res.results[c]["out"].astype(np.float32).reshape(BLOC, S, H))
    if _trace:
        return out, res
    return out

